# revision 58
# baseline (speedup 1.0000x reference)
"""Trainium2 Bass kernel for multi-head causal self-attention.

Problem: B=4, S=2048, D=768, H=12, DH=64 (fp32).
  Q = x @ W_Q + b_Q; K, V likewise
  scores = QK^T / sqrt(DH), causal mask, softmax
  out = (probs @ V) @ W_O + b_O

Sharding over 8 cores: core c -> batch b = c//2, head-half hh = c%2
(6 heads each). Fully local compute, no collectives; the two partial
outputs per batch (head-halves) are summed on the host during unshard.

Device layout is "transposed" everywhere (sequence on the free dim):
  xT   [D, S]       (host pre-transposes)
  QT,KT[384, S]     hk on partitions (3 chunks of 128 = 2 heads)
  V    [S, 453]     natural layout, per-head 65 cols (64 V + 1 ones col
                    so that P^T@[V|1] also accumulates softmax sums in the
                    same PSUM accumulation; PV uses a 128-wide lhsT window)
  S^T  [keys, q]    scores transposed -> softmax sum over keys is done
                    by the ones column in the PV matmul
  Z^T  [384, S]     normalized attention output
  outT [D, S]       host transposes back

Softmax skips the max-subtraction (scores are ~N(0, 0.3); exp is safe)
which is mathematically identical to the reference softmax.

Schedule: the TRN2 PE only reaches its top clock after ~3us of
*continuous* execution; any stall drops it back to half speed, so the
whole kernel is issued as one long interleave that keeps the PE stream
stall-free:
  stage 0: V projection for the first 6 key tiles + Q/K projection for
           head-pair 0, dt-major across the PSUM banks so the
           accumulation is paced by the x DMA stream; scratch "warm-up"
           matmuls fill the DMA gaps so the activity monitor grants the
           full PE rate early (it needs ~2 epochs of sustained duty)
  stages 1..3: attention for pair i, with a global FIFO of "filler"
           matmuls (V projection for key tiles 6-15, Q/K projections
           for pairs 1/2, W_O output chunks as their inputs complete)
           pumped between the score/PV matmuls so the PE never waits
           on the softmax exp (ACT engine).  PV for key tile kt is
           issued two steps behind its scores so the exp has a full
           pipeline stage of slack.
Scores for both heads of a pair accumulate into one PSUM tile so the
exp is a single fused ACT instruction per key tile.
"""

from collections import deque

import numpy as np

import concourse.mybir as mybir
import concourse.tile as tile
from concourse import bacc, bass_utils

F32 = mybir.dt.float32
BF16 = mybir.dt.bfloat16
CD = BF16

B, S, D, H, DH = 4, 2048, 768, 12, 64
HL = 6                # heads per core
HK = HL * DH          # 384
NPAIR = HL // 2       # 3 chunks of 2 heads (128 partitions each)
P = 128
NDT = D // P          # 6 d-tiles
NST = S // P          # 16 key tiles
QC = 512              # q chunk width (1 PSUM bank per head)
NQC = S // QC         # 4
VW = DH + 1           # 65 = V cols + ones col
SCALE = 1.0 / 8.0     # 1/sqrt(DH)

EXP = mybir.ActivationFunctionType.Exp


def _np_in(a):
    import ml_dtypes
    return np.ascontiguousarray(a, dtype=np.float32).astype(ml_dtypes.bfloat16)


def build_nc():
    nc = bacc.Bacc("TRN2", target_bir_lowering=False, debug=False, num_devices=8)

    xT = nc.dram_tensor("xT", [D, S], CD, kind="ExternalInput").ap()
    wq = nc.dram_tensor("wq", [D, HK], CD, kind="ExternalInput").ap()
    wk = nc.dram_tensor("wk", [D, HK], CD, kind="ExternalInput").ap()
    wv = nc.dram_tensor("wv", [D, HK], CD, kind="ExternalInput").ap()
    wo = nc.dram_tensor("wo", [HK, D], CD, kind="ExternalInput").ap()
    bq = nc.dram_tensor("bq", [HK], F32, kind="ExternalInput").ap()
    bk = nc.dram_tensor("bk", [HK], F32, kind="ExternalInput").ap()
    bv = nc.dram_tensor("bv", [HK], CD, kind="ExternalInput").ap()
    bo = nc.dram_tensor("bo", [D], F32, kind="ExternalInput").ap()
    # additive causal mask for the diagonal 128x128 block, [key, q] layout,
    # -1e4 where q < key; applied as iden.T @ mska inside the S accumulation
    mska = nc.dram_tensor("mska", [P, P], CD, kind="ExternalInput").ap()
    iden = nc.dram_tensor("iden", [P, P], CD, kind="ExternalInput").ap()
    # bf16 output halves the writeback DMA (the host sums the two
    # head-half partials in fp32; ~0.1% extra error vs the 2e-2 budget)
    out = nc.dram_tensor("out", [D, S], CD, kind="ExternalOutput").ap()

    with tile.TileContext(nc) as tc:
        with (
            tc.tile_pool(name="big", bufs=1) as big,
            tc.tile_pool(name="wts", bufs=1) as wts,
            tc.tile_pool(name="vpool", bufs=1) as vpool,
            tc.tile_pool(name="pp", bufs=3) as pp,
            tc.tile_pool(name="small", bufs=1) as small,
            tc.tile_pool(name="rcp", bufs=2) as rcp,
            tc.tile_pool(name="ot", bufs=4) as otp,
            tc.tile_pool(name="ps", bufs=1, space="PSUM") as ps,
        ):
            # ---- constants / small DMAs ---------------------------------
            mska_sb = small.tile([P, P], CD, tag="mska")
            nc.gpsimd.dma_start(out=mska_sb, in_=mska)
            iden_sb = small.tile([P, P], CD, tag="iden")
            nc.gpsimd.dma_start(out=iden_sb, in_=iden)
            # touch Exp once at t=0 so the ACT table load (~1.3us) overlaps
            # the input DMA phase instead of stalling the first real exp
            warm_sb = small.tile([1, 8], F32, tag="warm")
            nc.vector.memset(warm_sb, 1.0)
            nc.scalar.activation(warm_sb, warm_sb, EXP)
            bq_sb = small.tile([P, NPAIR], F32, tag="bq")
            nc.gpsimd.dma_start(out=bq_sb, in_=bq.rearrange("(c p) -> p c", p=P))
            bk_sb = small.tile([P, NPAIR], F32, tag="bk")
            nc.gpsimd.dma_start(out=bk_sb, in_=bk.rearrange("(c p) -> p c", p=P))
            bv_sb = small.tile([1, HK], CD, tag="bv")
            nc.gpsimd.dma_start(out=bv_sb, in_=bv.rearrange("(o k) -> o k", o=1))
            bvb = small.tile([P, HK], CD, tag="bvb")
            nc.gpsimd.partition_broadcast(bvb, bv_sb)
            bvv = bvb.rearrange("p (h c) -> p h c", c=DH)
            bo_sb = small.tile([P, NDT], F32, tag="bo")
            nc.gpsimd.dma_start(out=bo_sb, in_=bo.rearrange("(c p) -> p c", p=P))

            # PE warm-up: the activity monitor only grants the full-rate
            # state after ~2 epochs (3.4us each) of sustained PE duty; the
            # DMA-paced projection start otherwise leaves the core at half
            # rate for an unpredictable 10-25us.  Scratch matmuls fill the
            # head of the stream and the inter-d-tile DMA gaps.
            scr = small.tile([P, QC], CD, tag="scr")
            nc.vector.memset(scr, 0.125)

            def dummy(n):
                for _ in range(n):
                    dm = ps.tile([P, QC], F32, tag="proj", bufs=2, name="dm")
                    nc.tensor.matmul(dm, lhsT=scr[:, 0:P], rhs=scr, start=True, stop=True)

            dummy(4)

            # ---- x + weights; wq/wk staggered so they arrive just in
            # time for the pair-0 Q/K projection that follows V ----------
            xt = [big.tile([P, S], CD, tag=f"xt{dt}", name=f"xt{dt}") for dt in range(NDT)]
            wv_sb = [wts.tile([P, HK], CD, tag=f"wv{dt}", name=f"wv{dt}") for dt in range(NDT)]
            wq_sb = [wts.tile([P, HK], CD, tag=f"wq{dt}", name=f"wq{dt}") for dt in range(NDT)]
            wk_sb = [wts.tile([P, HK], CD, tag=f"wk{dt}", name=f"wk{dt}") for dt in range(NDT)]
            # three parallel DMA queues: sync + scalar are hardware-DGE
            # (~150GB/s each), gpsimd software-DGE (~67GB/s).  The x DMA
            # window is the stage-0 floor; wv must ride with x (the V
            # projection consumes both per d-tile), wq/wk are needed a few
            # us later, wo much later.
            for dt in range(NDT):
                nc.sync.dma_start(out=xt[dt][:, 0:1024], in_=xT[dt * P:(dt + 1) * P, 0:1024])
                nc.sync.dma_start(out=wv_sb[dt], in_=wv[dt * P:(dt + 1) * P, :])
                nc.scalar.dma_start(out=xt[dt][:, 1024:2048], in_=xT[dt * P:(dt + 1) * P, 1024:2048])
                nc.scalar.dma_start(out=wk_sb[dt], in_=wk[dt * P:(dt + 1) * P, :])
                nc.gpsimd.dma_start(out=wq_sb[dt], in_=wq[dt * P:(dt + 1) * P, :])
            wo_sb = []
            for c in range(NPAIR):
                t = wts.tile([P, D], CD, tag=f"wo{c}", name=f"wo{c}")
                nc.gpsimd.dma_start(out=t, in_=wo[c * P:(c + 1) * P, :])
                wo_sb.append(t)

            # ---- persistent SBUF tensors --------------------------------
            QT = [big.tile([P, S], CD, tag=f"qt{c}", name=f"qt{c}") for c in range(NPAIR)]
            KT = [big.tile([P, S], CD, tag=f"kt{c}", name=f"kt{c}") for c in range(NPAIR)]
            ZT = [big.tile([P, S], CD, tag=f"zt{c}", name=f"zt{c}") for c in range(NPAIR)]
            Vt = [vpool.tile([P, 516], CD, tag=f"v{st}", name=f"v{st}") for st in range(NST)]
            for st in range(NST):
                nc.gpsimd.memset(Vt[st][:, HL * VW:], 0.0)

            def v_copy_out(st, src):
                """PSUM -> Vt[st] with b_V add (DVE) + ones column."""
                vv = Vt[st][:, 0:HL * VW].rearrange("p (h c) -> p h c", c=VW)
                nc.vector.tensor_add(
                    vv[:, :, 0:DH],
                    src[:, 0:HK].rearrange("p (h c) -> p h c", c=DH),
                    bvv,
                )
                nc.gpsimd.memset(vv[:, :, DH:VW], 1.0)

            def psum_slots():
                """Eight 512-wide PSUM bank slots (the whole PSUM) for the
                dt-major projection phases."""
                s1 = ps.tile([P, 1024], F32, tag="sab", bufs=2, name="s1")
                s2 = ps.tile([P, 1024], F32, tag="sab", bufs=2, name="s2")
                o1 = ps.tile([P, 1024], F32, tag="oab", bufs=1, name="o1")
                p1 = ps.tile([P, QC], F32, tag="proj", bufs=2, name="p1")
                p2 = ps.tile([P, QC], F32, tag="proj", bufs=2, name="p2")
                return [s1[:, 0:512], s1[:, 512:1024], s2[:, 0:512],
                        s2[:, 512:1024], o1[:, 0:512], o1[:, 512:1024], p1, p2]

            # ---- stage 0a: V projection, key tiles 0-5 (DMA-paced) ------
            # natural layout [s, hk]; bias added on DVE during the PSUM
            # copy-out; each head gets a ones column for the softmax sums.
            # Only the sab/oab banks hold V groups so the proj banks stay
            # free for the warm-up scratch matmuls between d-tile rounds.
            s1 = ps.tile([P, 1024], F32, tag="sab", bufs=2, name="s1")
            s2 = ps.tile([P, 1024], F32, tag="sab", bufs=2, name="s2")
            o1 = ps.tile([P, 1024], F32, tag="oab", bufs=1, name="o1")
            vslots = [s1[:, 0:512], s1[:, 512:1024], s2[:, 0:512],
                      s2[:, 512:1024], o1[:, 0:512], o1[:, 512:1024]]
            for dt in range(NDT):
                for i in range(6):
                    nc.tensor.matmul(
                        vslots[i][:, 0:HK],
                        lhsT=xt[dt][:, i * P:(i + 1) * P],
                        rhs=wv_sb[dt],
                        start=(dt == 0),
                        stop=(dt == NDT - 1),
                    )
                if dt < NDT - 1:
                    dummy(2)
            for i in range(6):
                v_copy_out(i, vslots[i])

            # ---- stage 0b: Q/K projection for pair 0 (dt-major) ---------
            slots = psum_slots()
            for dt in range(NDT):
                for qc in range(NQC):
                    nc.tensor.matmul(
                        slots[qc],
                        lhsT=wq_sb[dt][:, 0:P],
                        rhs=xt[dt][:, qc * QC:(qc + 1) * QC],
                        start=(dt == 0),
                        stop=(dt == NDT - 1),
                    )
                for qc in range(NQC):
                    nc.tensor.matmul(
                        slots[4 + qc],
                        lhsT=wk_sb[dt][:, 0:P],
                        rhs=xt[dt][:, qc * QC:(qc + 1) * QC],
                        start=(dt == 0),
                        stop=(dt == NDT - 1),
                    )
            for qc in range(NQC):
                nc.scalar.add(QT[0][:, qc * QC:(qc + 1) * QC], slots[qc], bq_sb[:, 0:1])
                nc.scalar.add(KT[0][:, qc * QC:(qc + 1) * QC], slots[4 + qc], bk_sb[:, 0:1])

            # ---- PE filler generators -----------------------------------
            # each yields once per matmul; yields True on a chunk boundary
            # (a completed accumulation group) so consumers can force-drain
            # up to the chunk they depend on before emitting dependent work
            # (the PE executes in order -- a dependent instruction emitted
            # before its producer would deadlock the stream)
            def v_proj_gen():
                """V projection for key tiles 6-15."""
                for st in range(6, NST):
                    pt = ps.tile([P, QC], F32, tag="proj", bufs=2, name="vpt")
                    for dt in range(NDT):
                        nc.tensor.matmul(
                            pt[:, 0:HK],
                            lhsT=xt[dt][:, st * P:(st + 1) * P],
                            rhs=wv_sb[dt],
                            start=(dt == 0),
                            stop=(dt == NDT - 1),
                        )
                        yield dt == NDT - 1
                    v_copy_out(st, pt)

            def qk_proj_gen(pr, qcs=range(NQC)):
                """Q/K projection for pair pr."""
                for qc in qcs:
                    for w_sb, b_sb, dst in ((wq_sb, bq_sb, QT), (wk_sb, bk_sb, KT)):
                        pt = ps.tile([P, QC], F32, tag="proj", bufs=2, name="pjt")
                        for dt in range(NDT):
                            nc.tensor.matmul(
                                pt,
                                lhsT=w_sb[dt][:, pr * P:(pr + 1) * P],
                                rhs=xt[dt][:, qc * QC:(qc + 1) * QC],
                                start=(dt == 0),
                                stop=(dt == NDT - 1),
                            )
                            yield dt == NDT - 1
                        nc.scalar.add(dst[pr][:, qc * QC:(qc + 1) * QC], pt, b_sb[:, pr:pr + 1])

            def add_vec(o, i, b):
                nc.vector.tensor_scalar_add(o, i, b)

            def add_act(o, i, b):
                nc.scalar.add(o, i, b)

            def out_chunk(qc, dt, pt, add_fn, dma_eng):
                osb = otp.tile([P, QC], CD, tag="ot", bufs=4, name="osb")
                add_fn(osb, pt, bo_sb[:, dt:dt + 1])
                dma_eng.dma_start(
                    out=out[dt * P:(dt + 1) * P, qc * QC:(qc + 1) * QC],
                    in_=osb,
                )

            def wo_qc_gen(qc):
                """Output projection for one 512-wide q chunk."""
                for dt in range(NDT):
                    pt = ps.tile([P, QC], F32, tag="proj", bufs=2, name="wot")
                    for c in range(NPAIR):
                        nc.tensor.matmul(
                            pt,
                            lhsT=wo_sb[c][:, dt * P:(dt + 1) * P],
                            rhs=ZT[c][:, qc * QC:(qc + 1) * QC],
                            start=(c == 0),
                            stop=(c == NPAIR - 1),
                        )
                        yield c == NPAIR - 1
                    out_chunk(qc, dt, pt, add_vec, nc.sync)

            # ---- global filler FIFO -------------------------------------
            class Filler:
                def __init__(self, g, gate=0):
                    self.g = g
                    self.gate = gate
                    self.chunks = 0
                    self.done = False

                def step(self):
                    if self.done:
                        return False
                    try:
                        if next(self.g):
                            self.chunks += 1
                        return True
                    except StopIteration:
                        self.done = True
                        return False

                def ensure(self, k):
                    while self.chunks < k and not self.done:
                        self.step()

            fillers = deque()
            gstep = [0]

            def pump(n):
                while n > 0 and fillers:
                    f = fillers[0]
                    if f.done:
                        fillers.popleft()
                        continue
                    if f.gate > gstep[0]:
                        return
                    if f.step():
                        n -= 1

            # ---- attention ----------------------------------------------
            # ~1 filler matmul per step matches the ACT-vs-PE deficit;
            # higher rates drain the FIFO early and starve late chunks.
            # stage 3 pumps harder late because the W_O generators only
            # open mid-stage.
            PUMPS = {
                0: {0: 3, 1: 2, 2: 1, 3: 1},
                1: {0: 2, 1: 2, 2: 1, 3: 1},
                2: {0: 2, 1: 2, 2: 2, 3: 2},
            }

            def attention(pr, ensure=None):
                PUMP = PUMPS[pr]
                def emit_pv(kt, o, Pv, Ov, nkt):
                    for hh in range(2):
                        h65 = (2 * pr + hh) * VW
                        nc.tensor.matmul(
                            Ov[:, hh, o:QC],
                            lhsT=Vt[kt][:, h65:h65 + P],
                            rhs=Pv[:, hh, o:QC],
                            start=(kt == 0),
                            stop=(kt == nkt - 1),
                        )

                for qc in range(NQC):
                    if ensure is not None:
                        ensure(qc)
                    q0 = qc * QC
                    nkt = 4 * (qc + 1)
                    Oab = ps.tile([P, 1024], F32, tag="oab", bufs=1, name="oab")
                    Ov = Oab.rearrange("p (h c) -> p h c", h=2)
                    pend = deque()
                    for kt in range(nkt):
                        o = max(0, P * kt - q0)
                        diag = P * kt >= q0
                        Sab = ps.tile([P, 1024], F32, tag="sab", bufs=2, name="sab")
                        Sv = Sab.rearrange("p (h c) -> p h c", h=2)
                        for hh in range(2):
                            lo = hh * DH
                            nc.tensor.matmul(
                                Sv[:, hh, o:QC],
                                lhsT=KT[pr][lo:lo + DH, kt * P:(kt + 1) * P],
                                rhs=QT[pr][lo:lo + DH, q0 + o:q0 + QC],
                                start=True,
                                stop=not diag,
                            )
                        if diag:
                            for hh in range(2):
                                nc.tensor.matmul(
                                    Sv[:, hh, o:o + P],
                                    lhsT=iden_sb,
                                    rhs=mska_sb,
                                    start=False,
                                    stop=True,
                                )
                        # fused 2-head exp; softmax sums ride the V ones col
                        Pt = pp.tile([P, 1024], CD, tag="p", bufs=3, name="pt")
                        Pv = Pt.rearrange("p (h c) -> p h c", h=2)
                        nc.scalar.activation(Pv[:, :, o:QC], Sv[:, :, o:QC], EXP, scale=SCALE)
                        # PV trails its scores by two key tiles
                        if len(pend) == 2:
                            emit_pv(*pend.popleft(), Ov, nkt)
                        pend.append((kt, o, Pv))
                        pump(PUMP[qc])
                        gstep[0] += 1
                    while pend:
                        pump(3)
                        emit_pv(*pend.popleft(), Ov, nkt)

                    # normalize: ZT = O[0:64] * (1/sums); sums sit on
                    # partition 64.  One DVE copy frees the PSUM tile, the
                    # 32x32 stream transpose spreads the sums over 32
                    # partitions so the reciprocal runs wide, gpsimd
                    # broadcasts the result back over 64 partitions.
                    # The very last chunk of the kernel instead reads PSUM
                    # directly (no copy-out) and runs the chain per head
                    # (two pipelined 512-wide halves): this chain gates the
                    # final W_O matmuls, so its latency is the kernel tail.
                    last = (pr == NPAIR - 1 and qc == NQC - 1)
                    if last:
                        # per-head pipelined chains reading PSUM in place:
                        # this chain gates the final W_O matmuls, so its
                        # latency is the kernel tail
                        rbs = []
                        for hh in range(2):
                            cs = slice(hh * QC, (hh + 1) * QC)
                            tth = rcp.tile([32, QC], F32, tag=f"tth{hh}", bufs=1, name="tth")
                            nc.vector.transpose(tth, Oab[64:96, cs])
                            tvh = tth.rearrange("p (j c) -> p j c", c=32)[:, :, 0:1]
                            nc.vector.reciprocal(tvh, tvh)
                            rch = rcp.tile([32, QC], F32, tag=f"rch{hh}", bufs=1, name="rch")
                            nc.vector.transpose(rch, tth)
                            rbh = rcp.tile([DH, QC], F32, tag=f"rbh{hh}", bufs=1, name="rbh")
                            nc.gpsimd.partition_broadcast(rbh, rch[0:1, :])
                            rbs.append(rbh)
                        for hh in range(2):
                            nc.vector.tensor_mul(
                                ZT[pr][hh * DH:(hh + 1) * DH, q0:q0 + QC],
                                Oab[0:DH, hh * QC:(hh + 1) * QC],
                                rbs[hh],
                            )
                        continue
                    src = rcp.tile([96, 1024], F32, tag="ocp", bufs=2, name="ocp")
                    nc.vector.tensor_copy(src, Oab[0:96, :])
                    tt = rcp.tile([32, 1024], F32, tag="tt", bufs=2, name="tt")
                    nc.vector.transpose(tt, src[64:96, :])
                    tv = tt.rearrange("p (j c) -> p j c", c=32)[:, :, 0:1]
                    nc.vector.reciprocal(tv, tv)
                    rc32 = rcp.tile([32, 1024], F32, tag="rc32", bufs=2, name="rc32")
                    nc.vector.transpose(rc32, tt)
                    Rb = rcp.tile([DH, 1024], F32, tag="rb", bufs=2, name="rb")
                    nc.gpsimd.partition_broadcast(Rb, rc32[0:1, :])
                    for hh in range(2):
                        lo = hh * DH
                        nc.vector.tensor_mul(
                            ZT[pr][lo:lo + DH, q0:q0 + QC],
                            src[0:DH, hh * QC:(hh + 1) * QC],
                            Rb[:, hh * QC:(hh + 1) * QC],
                        )

            # stage 1: attention pair 0; filler = V tiles 8-15 then Q/K
            # pair 1.  Vt[st] is consumed at global step ~st+2, the 3/2
            # per-step early pump keeps V production well ahead of the PV
            # matmuls; the ensure() is a correctness backstop.
            vg = Filler(v_proj_gen())
            qk1 = Filler(qk_proj_gen(1))
            fillers.extend([vg, qk1])
            attention(0, ensure=lambda qc: vg.ensure(4 * (qc + 1) - 6))
            # stage 2: attention pair 1; filler = rest of Q/K pair 1 + the
            # first half of pair 2.  The second half of pair 2 is gated to
            # stage 3 so its first chunks (before the W_O gates open) have
            # PE filler too.
            qk2a = Filler(qk_proj_gen(2, (0, 1)))
            qk2b = Filler(qk_proj_gen(2, (2, 3)), gate=80)
            fillers.extend([qk2a, qk2b])
            attention(1, ensure=lambda qc: (vg.ensure(10), qk1.ensure(2 * (qc + 1))))
            # stage 3: attention pair 2 + W_O chunks once their q chunk's
            # ZT (all three pairs) is complete; pair-2 chunk qc finishes at
            # global step 80 + {4, 12, 24} -> gate a few steps later
            fillers.append(Filler(wo_qc_gen(0), gate=88))
            fillers.append(Filler(wo_qc_gen(1), gate=98))
            fillers.append(Filler(wo_qc_gen(2), gate=110))

            # last-resort filler: scratch matmuls keep the PE duty (and so
            # its full-rate grant) up through the final flush, where the
            # real filler supply runs dry
            def dummy_gen(n):
                for _ in range(n):
                    dm = ps.tile([P, QC], F32, tag="proj", bufs=2, name="dm")
                    nc.tensor.matmul(dm, lhsT=scr[:, 0:P], rhs=scr, start=True, stop=True)
                    yield True

            fillers.append(Filler(dummy_gen(16), gate=112))
            attention(2, ensure=lambda qc: (
                qk2a.ensure(2 * (qc + 1)) if qc < 2 else qk2b.ensure(2 * (qc - 1))
            ))
            pump(10 ** 6)  # drain any leftovers (gates are all past now)

            # ---- W_O tail for the last q chunk, c-major: the 12
            # pair-0/1 matmuls run while pair 2's last normalize drains.
            # Uses the sab/proj PSUM slots only -- the oab slot still holds
            # the last chunk's O tile (read in place by that normalize).
            # Scratch matmuls bridge the rest of that ~6.5us chain so the
            # PE keeps its full-rate grant for the final pair-2 matmuls.
            dummy(12)
            s1 = ps.tile([P, 1024], F32, tag="sab", bufs=2, name="ts1")
            s2 = ps.tile([P, 1024], F32, tag="sab", bufs=2, name="ts2")
            p1 = ps.tile([P, QC], F32, tag="proj", bufs=2, name="tp1")
            p2 = ps.tile([P, QC], F32, tag="proj", bufs=2, name="tp2")
            slots = [s1[:, 0:512], s1[:, 512:1024], s2[:, 0:512],
                     s2[:, 512:1024], p1, p2]
            for c in range(NPAIR):
                for dt in range(NDT):
                    nc.tensor.matmul(
                        slots[dt],
                        lhsT=wo_sb[c][:, dt * P:(dt + 1) * P],
                        rhs=ZT[c][:, 3 * QC:4 * QC],
                        start=(c == 0),
                        stop=(c == NPAIR - 1),
                    )
            tail_add = deque([add_vec, add_act])
            tail_dma = deque([nc.sync, nc.scalar])
            for dt in range(NDT):
                out_chunk(3, dt, slots[dt], tail_add[0], tail_dma[0])
                tail_add.rotate(-1)
                tail_dma.rotate(-1)

    nc.compile()
    return nc


_NC_CACHE = {}


def _get_nc():
    if "nc" not in _NC_CACHE:
        _NC_CACHE["nc"] = build_nc()
    return _NC_CACHE["nc"]


def make_in_maps(x, W_Q, W_K, W_V, W_O, b_Q, b_K, b_V, b_O):
    mask_add = np.tril(np.full((P, P), -1e4, np.float32), k=-1)
    identity = np.eye(P, dtype=np.float32)
    in_maps = []
    for c in range(8):
        b, hh = divmod(c, 2)
        hs = slice(HL * hh, HL * hh + HL)
        in_maps.append({
            "xT": _np_in(x[b].T),
            "wq": _np_in(W_Q[hs].transpose(1, 0, 2).reshape(D, HK)),
            "wk": _np_in(W_K[hs].transpose(1, 0, 2).reshape(D, HK)),
            "wv": _np_in(W_V[hs].transpose(1, 0, 2).reshape(D, HK)),
            "wo": _np_in(W_O[hs].reshape(HK, D)),
            "bq": np.ascontiguousarray(b_Q[hs].reshape(HK), np.float32),
            "bk": np.ascontiguousarray(b_K[hs].reshape(HK), np.float32),
            "bv": _np_in(b_V[hs].reshape(HK)),
            "bo": np.ascontiguousarray(b_O if hh == 0 else np.zeros(D), np.float32),
            "mska": _np_in(mask_add),
            "iden": _np_in(identity),
        })
    return in_maps


def run(inputs, trace=False):
    nc = _get_nc()
    in_maps = make_in_maps(**inputs)
    res = bass_utils.run_bass_kernel_spmd(
        nc, in_maps, core_ids=list(range(8)), trace=trace,
        **({"trace_cores": [0]} if trace else {}),
    )
    outs = [r["out"] for r in res.results]
    full = np.empty((B, S, D), np.float32)
    for b in range(B):
        full[b] = (outs[2 * b].astype(np.float32) + outs[2 * b + 1].astype(np.float32)).T
    return full, res


def kernel(**inputs):
    full, _ = run(inputs)
    return full
